# revision 5
# baseline (speedup 1.0000x reference)
"""Trainium2 Bass kernel for nn_GaussianUnit (spike whitening + MAP scale).

Strategy (data-parallel over spikes, per the sharding hint):
  - Host precomputes the 48 per-neighborhood whiteners Winv = chol(kron(T, Sg))^-1
    (tiny 50x50 factorizations) plus the derived per-group vectors:
      mu_g  = mean_full[:, nb_g].reshape(50)      (per-group mean)
      nu_g  = Winv_g @ mu_g                        (whitened mean, also an output row)
      q_g   = Winv_g^T @ nu_g                      (so <zw, nu> = <f, q> - <mu, q>)
  - Algebra folded into ONE augmented matmul per spike group:
      zw_i = Winv_g f_i - nu_g,   <zw_i, nu_g> = <f_i, q_g> - d0_g
    via lhsT_aug [51, 51] = [[Winv^T, q], [-nu, -d0]] acting on [f; 1].
  - Spikes are sorted by neighborhood on host and packed into 48 groups of
    static capacity CAP.  Two groups are stacked per matmul (block-diagonal
    102x102 lhsT) so the 128-partition contraction is well used.
  - 8 cores x 3 group-pairs each; each core runs plain fp32 matmuls over its
    [102, CAP] feature panels and writes [102, CAP] panels of (zw^T, dot).
  - Host unsorts zw, gathers nu = nu_g[gid], forms the per-spike Adam inputs
      ztnusq_i = (w_i <zw_i, nu_i>)^2,  nutnu_i = w_i^2 |nu_g|^2
    and runs the scalar Adam MAP recursion (identical math to the reference,
    with the reference's own early-freeze used as an early exit).

The heavy data movement (features in, zw panels out) runs on the 8 NeuronCores;
only O(G) tables and O(n) scalars are handled on host.
"""

import sys

for _p in ("/opt/trn_rl_repo", "/root/.axon_site/_ro/trn_rl_repo"):
    if _p not in sys.path:
        sys.path.append(_p)

import numpy as np

N, R, K, C, G = 50000, 5, 10, 384, 48
RK = R * K  # 50
RKA = RK + 1  # 51 (augmented with ones row)
ALPHA, BETA = 10.0, 100.0
N_CORES = 8
G_PER_CORE = G // N_CORES  # 6
PAIRS = G_PER_CORE // 2  # 3 (two groups stacked per matmul)
CAP = 1280  # static per-group spike capacity (seed-0 max count ~1100)
CHUNK = 512  # PSUM free-dim chunk (fp32 moving max / one bank)

_COMPILED = {}

# test-harness hook: set TRACE["enabled"]=True to capture an NTFF profile of
# the device run; results land in TRACE["last"] (BassKernelResults).
TRACE = {"enabled": False, "last": None}


def _build_bass():
    """Per-core Bass program: 3 pairs x ceil(CAP/CHUNK) augmented matmuls."""
    import concourse.bass as bass
    import concourse.tile as tile
    from concourse import bacc, mybir

    nc = bacc.Bacc("TRN2", target_bir_lowering=False, debug=False,
                   num_devices=N_CORES)
    fT = nc.dram_tensor("fT", [PAIRS, 2 * RKA, CAP], mybir.dt.float32,
                        kind="ExternalInput")
    wT = nc.dram_tensor("wT", [2 * RKA, PAIRS * 2 * RKA], mybir.dt.float32,
                        kind="ExternalInput")
    zwT = nc.dram_tensor("zwT", [PAIRS, 2 * RKA, CAP], mybir.dt.float32,
                         kind="ExternalOutput")

    P2 = 2 * RKA  # 102
    nchunk = (CAP + CHUNK - 1) // CHUNK

    with tile.TileContext(nc) as tc:
        with (
            tc.tile_pool(name="w", bufs=1) as wpool,
            tc.tile_pool(name="x", bufs=2) as xpool,
            tc.tile_pool(name="y", bufs=2) as ypool,
            tc.tile_pool(name="ps", bufs=4, space="PSUM") as pspool,
        ):
            wtile = wpool.tile([P2, PAIRS * P2], mybir.dt.float32)
            nc.sync.dma_start(out=wtile[:], in_=wT[:])
            for p in range(PAIRS):
                xtile = xpool.tile([P2, CAP], mybir.dt.float32)
                nc.sync.dma_start(out=xtile[:], in_=fT[p])
                ytile = ypool.tile([P2, CAP], mybir.dt.float32)
                for k in range(nchunk):
                    lo = k * CHUNK
                    sz = min(CHUNK, CAP - lo)
                    ps = pspool.tile([P2, CHUNK], mybir.dt.float32)
                    nc.tensor.matmul(
                        ps[:, :sz],
                        wtile[:, p * P2:(p + 1) * P2],
                        xtile[:, lo:lo + sz],
                        start=True, stop=True,
                    )
                    nc.vector.tensor_copy(ytile[:, lo:lo + sz], ps[:, :sz])
                nc.sync.dma_start(out=zwT[p], in_=ytile[:])
    nc.compile()
    return nc


def _get_compiled():
    if "nc" not in _COMPILED:
        _COMPILED["nc"] = _build_bass()
    return _COMPILED["nc"]


def _whitener_tables(mean_full, T, S, neighborhoods):
    """Host: Winv, nu, q, d0, |nu|^2 per neighborhood (float64 -> float32)."""
    from scipy.linalg import cholesky, solve_triangular

    T64 = np.asarray(T, np.float64)
    S64 = np.asarray(S, np.float64)
    mf64 = np.asarray(mean_full, np.float64)
    nb = np.asarray(neighborhoods)
    g = nb.shape[0]
    k = nb.shape[1]
    rk = T64.shape[0] * k
    Winv = np.empty((g, rk, rk), np.float64)
    mu = np.empty((g, rk), np.float64)
    I = np.eye(rk)
    for i in range(g):
        Sg = S64[np.ix_(nb[i], nb[i])]
        cov = np.kron(T64, Sg)
        L = cholesky(cov, lower=True)
        Winv[i] = solve_triangular(L, I, lower=True)
        mu[i] = mf64[:, nb[i]].reshape(rk)
    nu = np.einsum("gij,gj->gi", Winv, mu)
    q = np.einsum("gji,gj->gi", Winv, nu)
    d0 = np.einsum("gi,gi->g", mu, q)
    c = np.einsum("gi,gi->g", nu, nu)
    return (Winv.astype(np.float32), mu.astype(np.float32),
            nu.astype(np.float32), q.astype(np.float32),
            d0.astype(np.float32), c.astype(np.float32))


def _adam_scale(ztnusq, nutnu, n):
    """Scalar Adam MAP loop — same recursion as the reference, with the
    reference's done-freeze exploited as an early exit (exact)."""
    a_const = ALPHA + n / 2.0 - 1.0
    b1, b2, eps, lr, xtol = 0.9, 0.999, 1e-8, 0.1, 0.01
    av = np.asarray(nutnu, np.float64)
    bv = np.asarray(ztnusq, np.float64)
    t = 0.0
    m = 0.0
    v = 0.0
    for j in range(2000):
        lam = np.exp(t)
        npl = lam + av
        inv = 1.0 / npl
        s1 = inv.sum()
        s2 = (bv * inv * inv).sum()
        g = lam * (BETA + 0.5 * s1 + 0.5 * s2) - a_const
        m = b1 * m + (1.0 - b1) * g
        v = b2 * v + (1.0 - b2) * g * g
        jf = j + 1.0
        mhat = m / (1.0 - b1 ** jf)
        vhat = v / (1.0 - b2 ** jf)
        t1 = t - lr * mhat / (np.sqrt(vhat) + eps)
        if j > 8 and abs(np.exp(t1) - lam) < xtol:
            t = t1
            break
        t = t1
    return np.float32(1.0 / np.sqrt(np.exp(t)))


def kernel(features, mean_full, T, S, weights, neighborhood_ids, neighborhoods):
    features = np.asarray(features, np.float32)
    mean_full = np.asarray(mean_full, np.float32)
    weights = np.asarray(weights, np.float32)
    gid = np.asarray(neighborhood_ids).astype(np.int64)
    nb = np.asarray(neighborhoods).astype(np.int64)
    n = features.shape[0]
    r, k = features.shape[1], features.shape[2]
    rk = r * k
    g = nb.shape[0]

    Winv, mu, nu_t, q_t, d0_t, c_t = _whitener_tables(mean_full, T, S, nb)

    # ---- sort spikes by neighborhood, pack into static-capacity groups ----
    order = np.argsort(gid, kind="stable")
    counts = np.bincount(gid, minlength=g)
    starts = np.zeros(g + 1, np.int64)
    np.cumsum(counts, out=starts[1:])
    ff = features.reshape(n, rk)

    capped = np.minimum(counts, CAP)
    slot = np.arange(CAP)
    valid = slot[None, :] < capped[:, None]  # [g, CAP]
    idx = np.zeros((g, CAP), np.int64)
    for gi in range(g):
        idx[gi, :capped[gi]] = order[starts[gi]:starts[gi] + capped[gi]]

    # features panels: [g, rk, CAP], zero-padded, plus ones row
    FF = ff[idx] * valid[:, :, None]  # [g, CAP, rk]
    FFt = np.ascontiguousarray(FF.transpose(0, 2, 1))  # [g, rk, CAP]
    A = FFt.reshape(N_CORES, PAIRS, 2, rk, CAP)
    fT_in = np.zeros((N_CORES, PAIRS, 2 * RKA, CAP), np.float32)
    fT_in[:, :, 0:rk] = A[:, :, 0]
    fT_in[:, :, rk] = 1.0
    fT_in[:, :, RKA:RKA + rk] = A[:, :, 1]
    fT_in[:, :, RKA + rk] = 1.0

    # augmented block-diagonal weights per pair
    LT = np.zeros((g, RKA, RKA), np.float32)
    LT[:, 0:rk, 0:rk] = Winv.transpose(0, 2, 1)
    LT[:, 0:rk, rk] = q_t
    LT[:, rk, 0:rk] = -nu_t
    LT[:, rk, rk] = -d0_t
    W_in = np.zeros((N_CORES, 2 * RKA, PAIRS, 2 * RKA), np.float32)
    for gi in range(g):
        c_id, rest = divmod(gi, G_PER_CORE)
        p_id, half = divmod(rest, 2)
        o = RKA * half
        W_in[c_id, o:o + RKA, p_id, o:o + RKA] = LT[gi]

    # ---- run the Bass kernel on the 8 cores ----
    import os

    if os.environ.get("KERNEL_SIM", "0") == "1":
        # numpy stand-in for the device matmul (host-math debugging only)
        ZT = np.einsum("cpji,cpjs->cpis",
                       W_in.reshape(N_CORES, 2 * RKA, PAIRS, 2 * RKA)
                       .transpose(0, 2, 1, 3),
                       fT_in).astype(np.float32)
    else:
        from concourse.bass_utils import run_bass_kernel_spmd

        nc = _get_compiled()
        in_maps = [
            {"fT": fT_in[c_id],
             "wT": W_in[c_id].reshape(2 * RKA, PAIRS * 2 * RKA)}
            for c_id in range(N_CORES)
        ]
        kw = {}
        if TRACE["enabled"]:
            kw = dict(trace=True, trace_cores=list(range(N_CORES)))
        res = run_bass_kernel_spmd(nc, in_maps, core_ids=list(range(N_CORES)),
                                   **kw)
        TRACE["last"] = res
        ZT = np.stack([res.results[c_id]["zwT"] for c_id in range(N_CORES)])
    # [cores, PAIRS, 102, CAP] -> [g, RKA, CAP]
    Z2 = ZT.reshape(N_CORES, PAIRS, 2, RKA, CAP).reshape(g, RKA, CAP)

    zw_flat = np.empty((n, rk), np.float32)
    dot_flat = np.empty(n, np.float32)
    zw_sorted = Z2[:, 0:rk, :].transpose(0, 2, 1)  # [g, CAP, rk]
    zw_flat[idx[valid]] = zw_sorted[valid]
    dot_flat[idx[valid]] = Z2[:, rk, :][valid]

    # host fallback for any group overflowing the static capacity
    if np.any(counts > CAP):
        for gi in np.nonzero(counts > CAP)[0]:
            extra = order[starts[gi] + CAP:starts[gi] + counts[gi]]
            zf = ff[extra] @ Winv[gi].T - nu_t[gi]
            zw_flat[extra] = zf
            dot_flat[extra] = zf @ nu_t[gi]

    nu_out = nu_t[gid].reshape(n, r, k)
    zw_out = zw_flat.reshape(n, r, k)

    wdot = weights * dot_flat
    ztnusq = wdot * wdot
    nutnu = weights * weights * c_t[gid]
    scale = _adam_scale(ztnusq, nutnu, n)
    return zw_out, nu_out, scale


# revision 7
# speedup vs baseline: 1.6749x; 1.6749x over previous
"""Trainium2 Bass kernel for nn_GaussianUnit (spike whitening + MAP scale).

Strategy (data-parallel over spikes, per the sharding hint):
  - Host precomputes the 48 per-neighborhood whiteners Winv = chol(kron(T, Sg))^-1
    (tiny 50x50 factorizations) plus the derived per-group vectors:
      mu_g  = mean_full[:, nb_g].reshape(50)      (per-group mean)
      nu_g  = Winv_g @ mu_g                        (whitened mean, also an output row)
      q_g   = Winv_g^T @ nu_g                      (so <zw, nu> = <f, q> - <mu, q>)
  - Algebra folded into ONE augmented matmul per spike group:
      zw_i = Winv_g f_i - nu_g,   <zw_i, nu_g> = <f_i, q_g> - d0_g
    via lhsT_aug [51, 51] = [[Winv^T, q], [-nu, -d0]] acting on [f; 1].
  - Spikes are sorted by neighborhood on host and packed into 48 groups of
    static capacity CAP.  Two groups are stacked per matmul (block-diagonal
    102x102 lhsT) so the 128-partition contraction is well used.
  - 8 cores x 3 group-pairs each; each core runs plain fp32 matmuls over its
    [102, CAP] feature panels and writes [102, CAP] panels of (zw^T, dot).
  - Host unsorts zw, gathers nu = nu_g[gid], forms the per-spike Adam inputs
      ztnusq_i = (w_i <zw_i, nu_i>)^2,  nutnu_i = w_i^2 |nu_g|^2
    and runs the scalar Adam MAP recursion (identical math to the reference,
    with the reference's own early-freeze used as an early exit).

The heavy data movement (features in, zw panels out) runs on the 8 NeuronCores;
only O(G) tables and O(n) scalars are handled on host.
"""

import sys

for _p in ("/opt/trn_rl_repo", "/root/.axon_site/_ro/trn_rl_repo"):
    if _p not in sys.path:
        sys.path.append(_p)

import numpy as np

N, R, K, C, G = 50000, 5, 10, 384, 48
RK = R * K  # 50
RKA = RK + 1  # 51 (augmented with ones row)
ALPHA, BETA = 10.0, 100.0
N_CORES = 8
G_PER_CORE = G // N_CORES  # 6
PAIRS = G_PER_CORE // 2  # 3 (two groups stacked per matmul)
CAP = 1152  # static per-group spike capacity (seed-0 max count ~1100)
CHUNK = 384  # PSUM free-dim chunk (<=512 fp32 moving max / one bank)
F32R = True  # float32r matmul inputs: full-rate PE, ~tf32 multiply precision

_COMPILED = {}

# test-harness hook: set TRACE["enabled"]=True to capture an NTFF profile of
# the device run; results land in TRACE["last"] (BassKernelResults).
TRACE = {"enabled": False, "last": None}


def _build_bass():
    """Per-core Bass program: 3 pairs x ceil(CAP/CHUNK) augmented matmuls."""
    import concourse.bass as bass
    import concourse.tile as tile
    from concourse import bacc, mybir

    import os

    f32r = F32R and os.environ.get("KERNEL_F32R", "1") == "1"
    in_dt = mybir.dt.float32r if f32r else mybir.dt.float32

    nc = bacc.Bacc("TRN2", target_bir_lowering=False, debug=False,
                   num_devices=N_CORES)
    fT = nc.dram_tensor("fT", [PAIRS, 2 * RKA, CAP], in_dt,
                        kind="ExternalInput")
    wT = nc.dram_tensor("wT", [2 * RKA, PAIRS * 2 * RKA], in_dt,
                        kind="ExternalInput")
    zwT = nc.dram_tensor("zwT", [PAIRS, 2 * RKA, CAP], mybir.dt.float32,
                         kind="ExternalOutput")

    P2 = 2 * RKA  # 102
    nchunk = (CAP + CHUNK - 1) // CHUNK

    with tile.TileContext(nc) as tc:
        with (
            tc.tile_pool(name="w", bufs=1) as wpool,
            tc.tile_pool(name="x", bufs=3) as xpool,
            tc.tile_pool(name="y", bufs=3) as ypool,
            tc.tile_pool(name="ps", bufs=6, space="PSUM") as pspool,
        ):
            # all input panels first: they FIFO on the SP HWDGE ring while
            # the weights + output DMAs ride the ACT ring concurrently
            xtiles = []
            for p in range(PAIRS):
                xtile = xpool.tile([P2, CAP], in_dt, tag="x")
                nc.sync.dma_start(out=xtile[:], in_=fT[p])
                xtiles.append(xtile)
            wtile = wpool.tile([P2, PAIRS * P2], in_dt)
            nc.scalar.dma_start(out=wtile[:], in_=wT[:])
            for p in range(PAIRS):
                ytile = ypool.tile([P2, CAP], mybir.dt.float32, tag="y")
                for k in range(nchunk):
                    lo = k * CHUNK
                    sz = min(CHUNK, CAP - lo)
                    ps = pspool.tile([P2, CHUNK], mybir.dt.float32, tag="ps")
                    nc.tensor.matmul(
                        ps[:, :sz],
                        wtile[:, p * P2:(p + 1) * P2],
                        xtiles[p][:, lo:lo + sz],
                        start=True, stop=True,
                    )
                    nc.vector.tensor_copy(ytile[:, lo:lo + sz], ps[:, :sz])
                nc.scalar.dma_start(out=zwT[p], in_=ytile[:])
    nc.compile()
    return nc


def _get_compiled():
    if "nc" not in _COMPILED:
        _COMPILED["nc"] = _build_bass()
    return _COMPILED["nc"]


def _whitener_tables(mean_full, T, S, neighborhoods):
    """Host: Winv, nu, q, d0, |nu|^2 per neighborhood (float64 -> float32)."""
    from scipy.linalg import cholesky, solve_triangular

    T64 = np.asarray(T, np.float64)
    S64 = np.asarray(S, np.float64)
    mf64 = np.asarray(mean_full, np.float64)
    nb = np.asarray(neighborhoods)
    g = nb.shape[0]
    k = nb.shape[1]
    rk = T64.shape[0] * k
    Winv = np.empty((g, rk, rk), np.float64)
    mu = np.empty((g, rk), np.float64)
    I = np.eye(rk)
    for i in range(g):
        Sg = S64[np.ix_(nb[i], nb[i])]
        cov = np.kron(T64, Sg)
        L = cholesky(cov, lower=True)
        Winv[i] = solve_triangular(L, I, lower=True)
        mu[i] = mf64[:, nb[i]].reshape(rk)
    nu = np.einsum("gij,gj->gi", Winv, mu)
    q = np.einsum("gji,gj->gi", Winv, nu)
    d0 = np.einsum("gi,gi->g", mu, q)
    c = np.einsum("gi,gi->g", nu, nu)
    return (Winv.astype(np.float32), mu.astype(np.float32),
            nu.astype(np.float32), q.astype(np.float32),
            d0.astype(np.float32), c.astype(np.float32))


def _adam_scale(ztnusq, nutnu, n):
    """Scalar Adam MAP loop — same recursion as the reference, with the
    reference's done-freeze exploited as an early exit (exact)."""
    a_const = ALPHA + n / 2.0 - 1.0
    b1, b2, eps, lr, xtol = 0.9, 0.999, 1e-8, 0.1, 0.01
    av = np.asarray(nutnu, np.float64)
    bv = np.asarray(ztnusq, np.float64)
    t = 0.0
    m = 0.0
    v = 0.0
    for j in range(2000):
        lam = np.exp(t)
        npl = lam + av
        inv = 1.0 / npl
        s1 = inv.sum()
        s2 = (bv * inv * inv).sum()
        g = lam * (BETA + 0.5 * s1 + 0.5 * s2) - a_const
        m = b1 * m + (1.0 - b1) * g
        v = b2 * v + (1.0 - b2) * g * g
        jf = j + 1.0
        mhat = m / (1.0 - b1 ** jf)
        vhat = v / (1.0 - b2 ** jf)
        t1 = t - lr * mhat / (np.sqrt(vhat) + eps)
        if j > 8 and abs(np.exp(t1) - lam) < xtol:
            t = t1
            break
        t = t1
    return np.float32(1.0 / np.sqrt(np.exp(t)))


def kernel(features, mean_full, T, S, weights, neighborhood_ids, neighborhoods):
    features = np.asarray(features, np.float32)
    mean_full = np.asarray(mean_full, np.float32)
    weights = np.asarray(weights, np.float32)
    gid = np.asarray(neighborhood_ids).astype(np.int64)
    nb = np.asarray(neighborhoods).astype(np.int64)
    n = features.shape[0]
    r, k = features.shape[1], features.shape[2]
    rk = r * k
    g = nb.shape[0]

    Winv, mu, nu_t, q_t, d0_t, c_t = _whitener_tables(mean_full, T, S, nb)

    # ---- sort spikes by neighborhood, pack into static-capacity groups ----
    order = np.argsort(gid, kind="stable")
    counts = np.bincount(gid, minlength=g)
    starts = np.zeros(g + 1, np.int64)
    np.cumsum(counts, out=starts[1:])
    ff = features.reshape(n, rk)

    capped = np.minimum(counts, CAP)
    slot = np.arange(CAP)
    valid = slot[None, :] < capped[:, None]  # [g, CAP]
    idx = np.zeros((g, CAP), np.int64)
    for gi in range(g):
        idx[gi, :capped[gi]] = order[starts[gi]:starts[gi] + capped[gi]]

    # features panels: [g, rk, CAP], zero-padded, plus ones row
    FF = ff[idx] * valid[:, :, None]  # [g, CAP, rk]
    FFt = np.ascontiguousarray(FF.transpose(0, 2, 1))  # [g, rk, CAP]
    A = FFt.reshape(N_CORES, PAIRS, 2, rk, CAP)
    fT_in = np.zeros((N_CORES, PAIRS, 2 * RKA, CAP), np.float32)
    fT_in[:, :, 0:rk] = A[:, :, 0]
    fT_in[:, :, rk] = 1.0
    fT_in[:, :, RKA:RKA + rk] = A[:, :, 1]
    fT_in[:, :, RKA + rk] = 1.0

    # augmented block-diagonal weights per pair
    LT = np.zeros((g, RKA, RKA), np.float32)
    LT[:, 0:rk, 0:rk] = Winv.transpose(0, 2, 1)
    LT[:, 0:rk, rk] = q_t
    LT[:, rk, 0:rk] = -nu_t
    LT[:, rk, rk] = -d0_t
    W_in = np.zeros((N_CORES, 2 * RKA, PAIRS, 2 * RKA), np.float32)
    for gi in range(g):
        c_id, rest = divmod(gi, G_PER_CORE)
        p_id, half = divmod(rest, 2)
        o = RKA * half
        W_in[c_id, o:o + RKA, p_id, o:o + RKA] = LT[gi]

    # ---- run the Bass kernel on the 8 cores ----
    import os

    if os.environ.get("KERNEL_SIM", "0") == "1":
        # numpy stand-in for the device matmul (host-math debugging only)
        ZT = np.einsum("cpji,cpjs->cpis",
                       W_in.reshape(N_CORES, 2 * RKA, PAIRS, 2 * RKA)
                       .transpose(0, 2, 1, 3),
                       fT_in).astype(np.float32)
    else:
        from concourse.bass_utils import run_bass_kernel_spmd

        nc = _get_compiled()
        in_maps = [
            {"fT": fT_in[c_id],
             "wT": W_in[c_id].reshape(2 * RKA, PAIRS * 2 * RKA)}
            for c_id in range(N_CORES)
        ]
        kw = {}
        if TRACE["enabled"]:
            kw = dict(trace=True, trace_cores=list(range(N_CORES)))
        res = run_bass_kernel_spmd(nc, in_maps, core_ids=list(range(N_CORES)),
                                   **kw)
        TRACE["last"] = res
        ZT = np.stack([res.results[c_id]["zwT"] for c_id in range(N_CORES)])
    # [cores, PAIRS, 102, CAP] -> [g, RKA, CAP]
    Z2 = ZT.reshape(N_CORES, PAIRS, 2, RKA, CAP).reshape(g, RKA, CAP)

    zw_flat = np.empty((n, rk), np.float32)
    dot_flat = np.empty(n, np.float32)
    zw_sorted = Z2[:, 0:rk, :].transpose(0, 2, 1)  # [g, CAP, rk]
    zw_flat[idx[valid]] = zw_sorted[valid]
    dot_flat[idx[valid]] = Z2[:, rk, :][valid]

    # host fallback for any group overflowing the static capacity
    if np.any(counts > CAP):
        for gi in np.nonzero(counts > CAP)[0]:
            extra = order[starts[gi] + CAP:starts[gi] + counts[gi]]
            zf = ff[extra] @ Winv[gi].T - nu_t[gi]
            zw_flat[extra] = zf
            dot_flat[extra] = zf @ nu_t[gi]

    nu_out = nu_t[gid].reshape(n, r, k)
    zw_out = zw_flat.reshape(n, r, k)

    wdot = weights * dot_flat
    ztnusq = wdot * wdot
    nutnu = weights * weights * c_t[gid]
    scale = _adam_scale(ztnusq, nutnu, n)
    return zw_out, nu_out, scale
